# revision 1
# baseline (speedup 1.0000x reference)
"""Causal self-attention Trainium2 kernel (8 NeuronCores, bf16 compute).

Sharding: core c -> batch b = c//4, head group hg = c%4 (4 heads each).
Each core computes its heads' QKV projections, causal attention, and a
partial output projection yt[d, t] (transposed). Host sums the 4 partials
per batch, transposes, and adds b_proj.

Device dataflow per core:
  phase X : x [S,D] bf16 -> PE-transpose -> xT tiles [128d, S] resident
  per head: QT/KT/VT = W.T @ xT (transposed projections, hd on partitions)
            V = PE-transpose(VT)  (natural [tok, hd] layout)
            per q-span (512): for each k-block kj:
               ST[k,q] = KT_blk.T @ QT_span   (scores transposed, PSUM)
               += causal mask on diagonal blocks (DVE)
               PT = exp(scale*ST)             (ACT, bf16, unnormalized)
               sum[1,q]  += ones.T @ PT       (PE)
               OT[hd,q]  += V_blk.T @ PT      (PE)
            recipT = 1/sum (DVE), bcast = ones1.T @ recipT (PE rank-1, fp32)
            OT_sbuf = OT * bcast (DVE, bf16)
  proj    : yt[dc, t] += Wp_blk.T @ OT_h  accumulated over heads -> DRAM
"""
import numpy as np

B, S, D, H = 2, 2048, 2048, 16
HD = 128
NCORES = 8
HPC = H // (NCORES // B)     # heads per core = 4
NEG = -1e9


def build_nc(S=S, D=D, nh=HPC, span=512):
    import concourse.bass as bass
    import concourse.mybir as mybir
    from concourse import bacc
    from concourse.tile import TileContext

    f32 = mybir.dt.float32
    bf16 = mybir.dt.bfloat16
    KT = D // 128          # contraction tiles for qkv
    TT = S // 128          # token tiles
    NS = S // span         # q spans
    KPS = span // 128      # k-blocks per span
    scale = float(HD) ** -0.5

    nc = bacc.Bacc("TRN2", target_bir_lowering=False, debug=False)
    x_d = nc.dram_tensor("xt", [D, S], bf16, kind="ExternalInput").ap()
    wq_d = nc.dram_tensor("wqkv", [3 * nh * 128, D], bf16, kind="ExternalInput").ap()
    bq_d = nc.dram_tensor("bqkv", [128, 3 * nh], f32, kind="ExternalInput").ap()
    wp_d = nc.dram_tensor("wproj", [nh * 128, D], bf16, kind="ExternalInput").ap()
    tm_d = nc.dram_tensor("trimaskT", [128, 128], f32, kind="ExternalInput").ap()
    id_d = nc.dram_tensor("identb", [128, 128], bf16, kind="ExternalInput").ap()
    oc_d = nc.dram_tensor("ones_sq", [128, 128], bf16, kind="ExternalInput").ap()
    yt_d = nc.dram_tensor("yt", [D, S], f32, kind="ExternalOutput").ap()

    Act = mybir.ActivationFunctionType
    Alu = mybir.AluOpType

    with TileContext(nc) as tc:
        from contextlib import ExitStack
        with ExitStack() as ctx:
            res = ctx.enter_context(tc.tile_pool(name="res", bufs=1))
            w_p = ctx.enter_context(tc.tile_pool(name="w", bufs=4))
            wp_p = ctx.enter_context(tc.tile_pool(name="wp", bufs=nh))
            qk_p = ctx.enter_context(tc.tile_pool(name="qk", bufs=2))
            v_p = ctx.enter_context(tc.tile_pool(name="v", bufs=2))
            pt_p = ctx.enter_context(tc.tile_pool(name="pt", bufs=4))
            sm_p = ctx.enter_context(tc.tile_pool(name="sm", bufs=4))
            yst_p = ctx.enter_context(tc.tile_pool(name="yst", bufs=2))
            ps_t = ctx.enter_context(tc.tile_pool(name="ps_t", bufs=2, space="PSUM"))
            ps_mm = ctx.enter_context(tc.tile_pool(name="ps_mm", bufs=2, space="PSUM"))
            ps_st = ctx.enter_context(tc.tile_pool(name="ps_st", bufs=4, space="PSUM"))

            # constants
            trimaskT = res.tile([128, 128], f32, tag="trimaskT")
            identb = res.tile([128, 128], bf16, tag="identb")
            ones_sq = res.tile([128, 128], bf16, tag="ones_sq")
            bq = res.tile([128, 3 * nh], f32, tag="bq")
            nc.sync.dma_start(trimaskT, tm_d)
            nc.sync.dma_start(identb, id_d)
            nc.sync.dma_start(ones_sq, oc_d)
            nc.sync.dma_start(bq, bq_d)

            # preload the first two W stripes so qkv h0 isn't gated on them
            prew = {}
            for p in range(2):
                wt0 = w_p.tile([128, D], bf16, tag="w", name=f"w0_{p}")
                nc.sync.dma_start(wt0, wq_d[p * nh * 128:(p * nh + 1) * 128, :])
                prew[(0, p)] = wt0

            # ---- load host-pre-transposed x: xT[kt] [128d, S] stripes,
            # split in half-stripes across DMA rings so early kt land fast ----
            xT = [res.tile([128, S], bf16, tag=f"xt{kt}", name=f"xt{kt}")
                  for kt in range(KT)]
            hS = S // 2
            for kt in range(KT):
                for hh in range(2):
                    nc.sync.dma_start(
                        xT[kt][:, hh * hS:(hh + 1) * hS],
                        x_d[kt * 128:(kt + 1) * 128, hh * hS:(hh + 1) * hS])

            # ---- per-head OT accumulation ----
            OT = [res.tile([128, S], bf16, tag=f"ot{h}", name=f"ot{h}")
                  for h in range(nh)]
            for h in range(nh):
                # prefetch this head's three W stripes up front
                wts = []
                for p in range(3):
                    wt = prew.pop((h, p), None)
                    if wt is None:
                        wt = w_p.tile([128, D], bf16, tag="w",
                                      name=f"w{h}_{p}")
                        nc.sync.dma_start(
                            wt, wq_d[(p * nh + h) * 128:
                                     (p * nh + h + 1) * 128, :])
                    wts.append(wt)
                # qkv projections (transposed: [hd, tok])
                qkvT = []
                for p in range(3):
                    tag = ("qt", "kt_", "vt")[p]
                    dst = qk_p.tile([128, S], bf16, tag=tag)
                    hp = p * nh + h
                    wt = wts[p]
                    for sp in range(NS):
                        ps = ps_mm.tile([128, span], f32, tag="mm")
                        for kt in range(KT):
                            nc.tensor.matmul(
                                ps, wt[:, kt * 128:(kt + 1) * 128],
                                xT[kt][:, sp * span:(sp + 1) * span],
                                start=(kt == 0), stop=(kt == KT - 1))
                        nc.scalar.activation(
                            dst[:, sp * span:(sp + 1) * span], ps,
                            Act.Identity, bias=bq[:, hp:hp + 1], scale=1.0)
                    qkvT.append(dst)
                QT, KTt, VT = qkvT

                # V natural [tok, hd]: PE-transpose VT in groups of 4
                vh = v_p.tile([128, S], bf16, tag="v")
                for tg in range(0, TT, 4):
                    n = min(4, TT - tg)
                    ps = ps_t.tile([128, 512], bf16, tag="tp")
                    for j in range(n):
                        nc.tensor.transpose(
                            ps[:, j * 128:(j + 1) * 128],
                            VT[:, (tg + j) * 128:(tg + j + 1) * 128], identb)
                    nc.any.tensor_copy(
                        vh[:, tg * 128:(tg + n) * 128], ps[:, :n * 128])

                # attention per q-span
                for sp in range(NS):
                    nkj = KPS * (sp + 1)   # causal: k-blocks 0..nkj-1
                    ps_o = ps_mm.tile([128, span], f32, tag="mm")
                    ps_s = ps_st.tile([128, span], f32, tag="st")
                    pend = []  # (kj, pt, qoff) awaiting sum/av emission

                    def flush_one():
                        kj, pt, qoff = pend.pop(0)
                        nc.tensor.matmul(
                            ps_s[:, qoff:], ones_sq, pt[:, qoff:],
                            start=(kj == 0), stop=(kj == nkj - 1))
                        nc.tensor.matmul(
                            ps_o[:, qoff:], vh[:, kj * 128:(kj + 1) * 128],
                            pt[:, qoff:], start=(kj == 0), stop=(kj == nkj - 1))

                    for kj in range(nkj):
                        qoff = max(0, (kj - KPS * sp)) * 128
                        ps = ps_st.tile([128, span], f32, tag="st")
                        nc.tensor.matmul(
                            ps[:, qoff:], KTt[:, kj * 128:(kj + 1) * 128],
                            QT[:, sp * span + qoff:(sp + 1) * span],
                            start=True, stop=True)
                        if kj >= KPS * sp:  # diagonal block: causal mask
                            nc.vector.tensor_tensor(
                                out=ps[:, qoff:qoff + 128],
                                in0=ps[:, qoff:qoff + 128],
                                in1=trimaskT, op=Alu.add)
                        pt = pt_p.tile([128, span], bf16, tag="pt")
                        nc.scalar.activation(
                            pt[:, qoff:], ps[:, qoff:], Act.Exp, scale=scale)
                        pend.append((kj, pt, qoff))
                        if len(pend) > 2:
                            flush_one()
                    while pend:
                        flush_one()

                    recipb = sm_p.tile([128, span], f32, tag="recipb")
                    nc.vector.reciprocal_approx_fast(out=recipb, in_=ps_s)
                    nc.vector.tensor_tensor(
                        out=OT[h][:, sp * span:(sp + 1) * span],
                        in0=ps_o, in1=recipb, op=Alu.mult)

            # ---- output projection: yt[dc, t] = sum_h Wp_h.T @ OT_h ----
            wp = []
            for h in range(nh):
                w = wp_p.tile([128, D], bf16, tag="wpt")
                nc.sync.dma_start(w, wp_d[h * 128:(h + 1) * 128, :])
                wp.append(w)
            for dc in range(D // 128):
                yst = yst_p.tile([128, S], f32, tag="yst")
                for sp in range(NS):
                    ps = ps_mm.tile([128, span], f32, tag="mm")
                    for h in range(nh):
                        nc.tensor.matmul(
                            ps, wp[h][:, dc * 128:(dc + 1) * 128],
                            OT[h][:, sp * span:(sp + 1) * span],
                            start=(h == 0), stop=(h == nh - 1))
                    nc.any.tensor_copy(yst[:, sp * span:(sp + 1) * span], ps)
                nc.sync.dma_start(yt_d[dc * 128:(dc + 1) * 128, :], yst)

    nc.finalize()
    return nc


def _prep_core_inputs(x, W_qkv, b_qkv, W_proj, core, S=S, D=D, nh=HPC):
    import ml_dtypes
    bf16 = ml_dtypes.bfloat16
    ngr = NCORES // B
    b, hg = core // ngr, core % ngr
    KT = D // 128
    Dfull = W_qkv.shape[0]

    wq = np.empty((3 * nh * 128, D), dtype=bf16)
    bq = np.zeros((128, 3 * nh), dtype=np.float32)
    for p in range(3):
        for h in range(nh):
            g = hg * nh + h
            col = p * Dfull + g * 128
            blk = W_qkv[:, col:col + 128]            # [D, 128]
            hp = p * nh + h
            wq[hp * 128:(hp + 1) * 128] = (
                blk.reshape(KT, 128, 128).transpose(1, 0, 2).reshape(128, D)
                .astype(bf16))
            bq[:, hp] = b_qkv[col:col + 128]
    wp = W_proj[hg * nh * 128:(hg + 1) * nh * 128, :].astype(bf16)

    r = np.arange(128)
    trimaskT = np.where(r[:, None] <= r[None, :], 0.0, NEG).astype(np.float32)
    return {
        "xt": np.ascontiguousarray(x[b].T).astype(bf16),
        "wqkv": wq,
        "bqkv": bq,
        "wproj": wp,
        "trimaskT": trimaskT,
        "identb": np.eye(128, dtype=bf16),
        "ones_sq": np.ones((128, 128), dtype=bf16),
    }


_CACHE = {}


def kernel(x, W_qkv, b_qkv, W_proj, b_proj, mask):
    from concourse.bass_utils import run_bass_kernel_spmd

    x = np.asarray(x)
    W_qkv = np.asarray(W_qkv)
    b_qkv = np.asarray(b_qkv)
    W_proj = np.asarray(W_proj)
    b_proj = np.asarray(b_proj)

    if "nc" not in _CACHE:
        _CACHE["nc"] = build_nc()
    nc = _CACHE["nc"]

    in_maps = [_prep_core_inputs(x, W_qkv, b_qkv, W_proj, c)
               for c in range(NCORES)]
    res = run_bass_kernel_spmd(nc, in_maps, core_ids=list(range(NCORES)))

    ngr = NCORES // B
    out = np.empty((B, S, D), dtype=np.float32)
    for b in range(B):
        acc = res.results[b * ngr]["yt"].astype(np.float32)
        for g in range(1, ngr):
            acc = acc + res.results[b * ngr + g]["yt"]
        out[b] = acc.T + b_proj[None, :]
    return out



# revision 3
# speedup vs baseline: 1.1023x; 1.1023x over previous
"""Causal self-attention Trainium2 kernel (8 NeuronCores, bf16 compute).

Sharding: core c -> batch b = c//4, head group hg = c%4 (4 heads each).
Each core computes its heads' QKV projections, causal attention, and a
partial output projection yt[d, t] (transposed). Host sums the 4 partials
per batch, transposes, and adds b_proj.

Device dataflow per core (software-pipelined across heads):
  qkv(h) : per span/proj: PSUM = W.T @ xT chunks -> ACT bias -> QT/KT/VT
           VT 128-blocks transposed to natural V via DMA XBAR transpose
  attn(h): per q-span (512): for each k-block kj:
             ST[k,q] = KT_blk.T @ QT_span   (PE, scores transposed)
             += causal mask on diagonal blocks (DVE)
             PT = exp(scale*ST)             (ACT, bf16, unnormalized)
             acc[128,q] += PT               (DVE, f32)
             OT[hd,q] += V_blk.T @ PT       (PE, lagged)
           sum = partition_all_reduce(acc)  (GPSIMD)
           recip (DVE), OT_sbuf = OT * recip (DVE, bf16)
  proj   : yt[dc,t] = sum_h Wp_h.T @ OT_h -> chunked DMA out
  Interleave: attn(h-1) units are woven between qkv(h) units, and
  attn(3) between proj units, so the ACT-bound exp chain never stalls
  the PE.
"""
import numpy as np

B, S, D, H = 2, 2048, 2048, 16
HD = 128
NCORES = 8
HPC = H // (NCORES // B)     # heads per core = 4
NEG = -1e9


def build_nc(S=S, D=D, nh=HPC, span=512):
    import concourse.mybir as mybir
    from concourse import bacc
    from concourse import bass_isa
    from concourse.tile import TileContext

    f32 = mybir.dt.float32
    bf16 = mybir.dt.bfloat16
    KT = D // 128          # contraction tiles for qkv
    TT = S // 128          # token tiles
    NS = S // span         # q spans
    KPS = span // 128      # k-blocks per span
    DC = D // 128
    scale = float(HD) ** -0.5
    LAG = 2

    nc = bacc.Bacc("TRN2", target_bir_lowering=False, debug=False)
    x_d = nc.dram_tensor("xt", [D, S], bf16, kind="ExternalInput").ap()
    wq_d = nc.dram_tensor("wqkv", [3 * nh * 128, D], bf16,
                          kind="ExternalInput").ap()
    bq_d = nc.dram_tensor("bqkv", [128, 3 * nh], f32, kind="ExternalInput").ap()
    wp_d = nc.dram_tensor("wproj", [nh * 128, D], bf16,
                          kind="ExternalInput").ap()
    tm_d = nc.dram_tensor("trimaskT", [128, 128], f32,
                          kind="ExternalInput").ap()
    yt_d = nc.dram_tensor("yt", [D, S], f32, kind="ExternalOutput").ap()

    Act = mybir.ActivationFunctionType
    Alu = mybir.AluOpType

    with TileContext(nc) as tc:
        from contextlib import ExitStack
        with ExitStack() as ctx:
            res = ctx.enter_context(tc.tile_pool(name="res", bufs=1))
            w_p = ctx.enter_context(tc.tile_pool(name="w", bufs=6))
            wp_p = ctx.enter_context(tc.tile_pool(name="wp", bufs=nh))
            qk_p = ctx.enter_context(tc.tile_pool(name="qk", bufs=2))
            v_p = ctx.enter_context(tc.tile_pool(name="v", bufs=2))
            pt_p = ctx.enter_context(tc.tile_pool(name="pt", bufs=4))
            sm_p = ctx.enter_context(tc.tile_pool(name="sm", bufs=2))
            yc_p = ctx.enter_context(tc.tile_pool(name="yc", bufs=4))
            ps_mm = ctx.enter_context(
                tc.tile_pool(name="ps_mm", bufs=2, space="PSUM"))
            ps_o = ctx.enter_context(
                tc.tile_pool(name="ps_o", bufs=2, space="PSUM"))
            ps_st = ctx.enter_context(
                tc.tile_pool(name="ps_st", bufs=4, space="PSUM"))

            # constants
            trimaskT = res.tile([128, 128], f32, tag="trimaskT")
            bq = res.tile([128, 3 * nh], f32, tag="bq")
            nc.sync.dma_start(trimaskT, tm_d)
            nc.sync.dma_start(bq, bq_d)

            wt = {}

            def issue_w(h, halves=1):
                for p in range(3):
                    t = w_p.tile([128, D], bf16, tag="w", name=f"w{h}_{p}")
                    r0 = (p * nh + h) * 128
                    hD = D // halves
                    for q in range(halves):
                        nc.sync.dma_start(
                            t[:, q * hD:(q + 1) * hD],
                            wq_d[r0:r0 + 128, q * hD:(q + 1) * hD])
                    wt[(h, p)] = t

            issue_w(0, halves=2)

            # ---- x resident as one big tile; panel-major (span-major) DMA
            # so qkv(0) span 0 can start before the rest of x lands ----
            xT = res.tile([128, KT * S], bf16, tag="xT")
            xT3 = xT.rearrange("p (kt s) -> p kt s", kt=KT)

            def xs(kt):
                return xT3[:, kt, :]

            for sp in range(NS):
                g = 2 if sp == 0 else 4
                g = min(g, KT)
                for k0 in range(0, KT, g):
                    src = x_d[k0 * 128:(k0 + g) * 128,
                              sp * span:(sp + 1) * span]
                    nc.sync.dma_start(
                        xT3[:, k0:k0 + g, sp * span:(sp + 1) * span],
                        src.rearrange("(kt p) s -> p kt s", p=128))

            # ---- persistent per-head outputs ----
            OT = [res.tile([128, S], bf16, tag=f"ot{h}", name=f"ot{h}")
                  for h in range(nh)]
            dsts = {}
            vblk = {}
            wpt = []

            def qkv_units(h):
                """Units of one span x one projection (KT matmuls + bias)."""
                if h + 1 < nh:
                    issue_w(h + 1)
                if h == nh - 1:
                    for g in range(nh):
                        t = wp_p.tile([128, D], bf16, tag="wpt",
                                      name=f"wp{g}")
                        nc.sync.dma_start(t, wp_d[g * 128:(g + 1) * 128, :])
                        wpt.append(t)
                QT = qk_p.tile([128, S], bf16, tag="qt", name=f"qt{h}")
                KTt = qk_p.tile([128, S], bf16, tag="kt_", name=f"ktt{h}")
                VT = qk_p.tile([128, S], bf16, tag="vt", name=f"vt{h}")
                dsts[h] = (QT, KTt)
                dst3 = (QT, KTt, VT)
                for sp in range(NS):
                    for p in range(3):
                        ps = ps_mm.tile([128, span], f32, tag="mm")
                        w = wt[(h, p)]
                        for kt in range(KT):
                            nc.tensor.matmul(
                                ps, w[:, kt * 128:(kt + 1) * 128],
                                xs(kt)[:, sp * span:(sp + 1) * span],
                                start=(kt == 0), stop=(kt == KT - 1))
                        hp = p * nh + h
                        nc.scalar.activation(
                            dst3[p][:, sp * span:(sp + 1) * span], ps,
                            Act.Identity, bias=bq[:, hp:hp + 1], scale=1.0)
                        if p == 2:
                            # V natural layout via DMA XBAR transpose
                            for j in range(KPS):
                                tb = sp * KPS + j
                                vb = v_p.tile([128, 128], bf16, tag=f"v{tb}",
                                              name=f"v{h}_{tb}")
                                vblk[(h, tb)] = vb
                                nc.sync.dma_start_transpose(
                                    vb, VT[:, tb * 128:(tb + 1) * 128])
                        yield (16, None)

            def attn_units(h):
                """Units of one k-block (score mm + exp + acc; lagged AV mm).
                Yields (weight, completed_span|None)."""
                QT, KTt = dsts[h]
                for sp in range(NS):
                    nkj = KPS * (sp + 1)
                    po = ps_o.tile([128, span], f32, tag="o")
                    acc = sm_p.tile([128, span], f32, tag="acc")
                    pend = []

                    def av(it):
                        kj, pt, qoff = it
                        nc.tensor.matmul(
                            po[:, qoff:], vblk[(h, kj)], pt[:, qoff:],
                            start=(kj == 0), stop=(kj == nkj - 1))

                    for kj in range(nkj):
                        while len(pend) > LAG:
                            av(pend.pop(0))
                        qoff = max(0, kj - KPS * sp) * 128
                        ps = ps_st.tile([128, span], f32, tag="st")
                        nc.tensor.matmul(
                            ps[:, qoff:], KTt[:, kj * 128:(kj + 1) * 128],
                            QT[:, sp * span + qoff:(sp + 1) * span],
                            start=True, stop=True)
                        if kj >= KPS * sp:  # diagonal block: causal mask
                            nc.vector.tensor_tensor(
                                out=ps[:, qoff:qoff + 128],
                                in0=ps[:, qoff:qoff + 128],
                                in1=trimaskT, op=Alu.add)
                        pt = pt_p.tile([128, span], bf16, tag="pt")
                        nc.scalar.activation(
                            pt[:, qoff:], ps[:, qoff:], Act.Exp, scale=scale)
                        if kj == 0:
                            nc.vector.tensor_copy(out=acc, in_=pt)
                        else:
                            nc.vector.tensor_tensor(
                                out=acc[:, qoff:], in0=acc[:, qoff:],
                                in1=pt[:, qoff:], op=Alu.add)
                        pend.append((kj, pt, qoff))
                        yield (2, None)
                    while pend:
                        av(pend.pop(0))
                    # softmax denominator entirely off the PE
                    sumb = sm_p.tile([128, span], f32, tag="sum")
                    nc.gpsimd.partition_all_reduce(
                        sumb, acc, channels=128,
                        reduce_op=bass_isa.ReduceOp.add)
                    recipb = sm_p.tile([128, span], f32, tag="recipb")
                    nc.vector.reciprocal_approx_fast(out=recipb, in_=sumb)
                    nc.vector.tensor_tensor(
                        out=OT[h][:, sp * span:(sp + 1) * span],
                        in0=po, in1=recipb, op=Alu.mult)
                    yield (1, sp)

            def drive_pair(ga, atot, gb, btot):
                """Weighted-fair merge of two unit generators."""
                aw = bw = 0.0
                adone = bdone = False
                while not (adone and bdone):
                    pick_a = (not adone) and (bdone or aw * btot <= bw * atot)
                    g = ga if pick_a else gb
                    try:
                        w, _ = next(g)
                        if pick_a:
                            aw += w
                        else:
                            bw += w
                    except StopIteration:
                        if pick_a:
                            adone = True
                        else:
                            bdone = True

            # ---- pipeline driver ----
            for _ in qkv_units(0):
                pass
            qkv_tot = 16 * 3 * NS
            attn_tot = 2 * (KPS * NS * (NS + 1) // 2) + NS
            for h in range(1, nh):
                drive_pair(qkv_units(h), qkv_tot, attn_units(h - 1), attn_tot)

            # ---- final phase: proj interleaved with attn(nh-1),
            # gated so proj of span sp waits for attn span sp ----
            punits = [(sp, dcp, half) for sp in range(NS)
                      for dcp in range(DC // 2) for half in range(2)]
            yc_cur = [None]

            def emit_proj(sp, dcp, half):
                if half == 0:
                    yc_cur[0] = yc_p.tile([128, 2 * span], f32, tag="yc",
                                          name=f"yc{sp}_{dcp}")
                yc = yc_cur[0]
                dc = dcp * 2 + half
                ps = ps_mm.tile([128, span], f32, tag="mm")
                for g in range(nh):
                    nc.tensor.matmul(
                        ps, wpt[g][:, dc * 128:(dc + 1) * 128],
                        OT[g][:, sp * span:(sp + 1) * span],
                        start=(g == 0), stop=(g == nh - 1))
                if half == 0:
                    nc.scalar.copy(yc[:, :span], ps)
                else:
                    nc.vector.tensor_copy(out=yc[:, span:], in_=ps)
                    dst = yt_d[dcp * 256:(dcp + 1) * 256,
                               sp * span:(sp + 1) * span]
                    nc.sync.dma_start(
                        dst.rearrange("(two p) s -> p two s", p=128),
                        yc.rearrange("p (two s) -> p two s", two=2))

            ag = attn_units(nh - 1)
            aw = pw = 0.0
            adone = False
            span_done = -1
            pi = 0
            ptot = 4 * len(punits)
            while (not adone) or pi < len(punits):
                can_p = pi < len(punits) and punits[pi][0] <= span_done
                pick_a = (not adone) and (
                    not can_p or aw * ptot <= pw * attn_tot)
                if pick_a:
                    try:
                        w, m = next(ag)
                        aw += w
                        if m is not None:
                            span_done = m
                    except StopIteration:
                        adone = True
                elif can_p:
                    emit_proj(*punits[pi])
                    pi += 1
                    pw += 4
                else:
                    # attn exhausted but gate not open: shouldn't happen
                    raise AssertionError("proj gating stuck")

    nc.finalize()
    return nc


def _prep_core_inputs(x, W_qkv, b_qkv, W_proj, core, S=S, D=D, nh=HPC):
    import ml_dtypes
    bf16 = ml_dtypes.bfloat16
    ngr = NCORES // B
    b, hg = core // ngr, core % ngr
    KT = D // 128
    Dfull = W_qkv.shape[0]

    wq = np.empty((3 * nh * 128, D), dtype=bf16)
    bq = np.zeros((128, 3 * nh), dtype=np.float32)
    for p in range(3):
        for h in range(nh):
            g = hg * nh + h
            col = p * Dfull + g * 128
            blk = W_qkv[:, col:col + 128]            # [D, 128]
            hp = p * nh + h
            wq[hp * 128:(hp + 1) * 128] = (
                blk.reshape(KT, 128, 128).transpose(1, 0, 2).reshape(128, D)
                .astype(bf16))
            bq[:, hp] = b_qkv[col:col + 128]
    wp = W_proj[hg * nh * 128:(hg + 1) * nh * 128, :].astype(bf16)

    r = np.arange(128)
    trimaskT = np.where(r[:, None] <= r[None, :], 0.0, NEG).astype(np.float32)
    return {
        "xt": np.ascontiguousarray(x[b].T).astype(bf16),
        "wqkv": wq,
        "bqkv": bq,
        "wproj": wp,
        "trimaskT": trimaskT,
    }


_CACHE = {}


def kernel(x, W_qkv, b_qkv, W_proj, b_proj, mask):
    from concourse.bass_utils import run_bass_kernel_spmd

    x = np.asarray(x)
    W_qkv = np.asarray(W_qkv)
    b_qkv = np.asarray(b_qkv)
    W_proj = np.asarray(W_proj)
    b_proj = np.asarray(b_proj)

    if "nc" not in _CACHE:
        _CACHE["nc"] = build_nc()
    nc = _CACHE["nc"]

    in_maps = [_prep_core_inputs(x, W_qkv, b_qkv, W_proj, c)
               for c in range(NCORES)]
    res = run_bass_kernel_spmd(nc, in_maps, core_ids=list(range(NCORES)))

    ngr = NCORES // B
    out = np.empty((B, S, D), dtype=np.float32)
    for b in range(B):
        acc = res.results[b * ngr]["yt"].astype(np.float32)
        for g in range(1, ngr):
            acc = acc + res.results[b * ngr + g]["yt"]
        out[b] = acc.T + b_proj[None, :]
    return out


# revision 11
# speedup vs baseline: 1.1049x; 1.0023x over previous
"""Causal self-attention Trainium2 kernel (8 NeuronCores, bf16 compute).

Sharding: core c -> batch b = c//4, head group hg = c%4 (4 heads each).
Each core computes its heads' QKV projections, causal attention, and a
partial output projection yt[d, t] (transposed). Host sums the 4 partials
per batch, transposes, and adds b_proj.

Device dataflow per core (software-pipelined across heads):
  qkv(h) : per span/proj: PSUM = W.T @ xT chunks -> ACT bias -> QT/KT/VT
           VT 128-blocks transposed to natural V via DMA XBAR transpose
  attn(h): per q-span (512): for each k-block kj:
             ST[k,q] = KT_blk.T @ QT_span   (PE, scores transposed)
             += causal mask on diagonal blocks (DVE)
             PT = exp(scale*ST)             (ACT, bf16, unnormalized)
             acc[128,q] += PT               (DVE, f32)
             OT[hd,q] += V_blk.T @ PT       (PE, lagged)
           sum = partition_all_reduce(acc)  (GPSIMD)
           recip (DVE), OT_sbuf = OT * recip (DVE, bf16)
  proj   : yt[dc,t] = sum_h Wp_h.T @ OT_h -> chunked DMA out
  Interleave: attn(h-1) units are woven between qkv(h) units, and
  attn(3) between proj units, so the ACT-bound exp chain never stalls
  the PE.
"""
import numpy as np

B, S, D, H = 2, 2048, 2048, 16
HD = 128
NCORES = 8
HPC = H // (NCORES // B)     # heads per core = 4
NEG = -1e9


def build_nc(S=S, D=D, nh=HPC, span=512):
    import concourse.mybir as mybir
    from concourse import bacc
    from concourse import bass_isa
    from concourse.tile import TileContext

    f32 = mybir.dt.float32
    bf16 = mybir.dt.bfloat16
    KT = D // 128          # contraction tiles for qkv
    TT = S // 128          # token tiles
    NS = S // span         # q spans
    KPS = span // 128      # k-blocks per span
    DC = D // 128
    scale = float(HD) ** -0.5
    LAG = 2

    nc = bacc.Bacc("TRN2", target_bir_lowering=False, debug=False)
    x_d = nc.dram_tensor("xt", [D, S], bf16, kind="ExternalInput").ap()
    wq_d = nc.dram_tensor("wqkv", [3 * nh * 128, D], bf16,
                          kind="ExternalInput").ap()
    bq_d = nc.dram_tensor("bqkv", [128, 3 * nh], f32, kind="ExternalInput").ap()
    wp_d = nc.dram_tensor("wproj", [nh * 128, D], bf16,
                          kind="ExternalInput").ap()
    tm_d = nc.dram_tensor("trimaskT", [128, 128], f32,
                          kind="ExternalInput").ap()
    id_d = nc.dram_tensor("identb", [128, 128], bf16, kind="ExternalInput").ap()
    yt_d = nc.dram_tensor("yt", [D, S], f32, kind="ExternalOutput").ap()

    Act = mybir.ActivationFunctionType
    Alu = mybir.AluOpType

    with TileContext(nc) as tc:
        from contextlib import ExitStack
        with ExitStack() as ctx:
            res = ctx.enter_context(tc.tile_pool(name="res", bufs=1))
            w_p = ctx.enter_context(tc.tile_pool(name="w", bufs=6))
            wp_p = ctx.enter_context(tc.tile_pool(name="wp", bufs=nh))
            qk_p = ctx.enter_context(tc.tile_pool(name="qk", bufs=2))
            v_p = ctx.enter_context(tc.tile_pool(name="v", bufs=2))
            pt_p = ctx.enter_context(tc.tile_pool(name="pt", bufs=4))
            sm_p = ctx.enter_context(tc.tile_pool(name="sm", bufs=2))
            yc_p = ctx.enter_context(tc.tile_pool(name="yc", bufs=4))
            ps_mm = ctx.enter_context(
                tc.tile_pool(name="ps_mm", bufs=2, space="PSUM"))
            ps_o = ctx.enter_context(
                tc.tile_pool(name="ps_o", bufs=2, space="PSUM"))
            ps_st = ctx.enter_context(
                tc.tile_pool(name="ps_st", bufs=4, space="PSUM"))

            # constants (issued on the ACT queue; SP queue is for x panels)
            trimaskT = res.tile([128, 128], f32, tag="trimaskT")
            identb = res.tile([128, 128], bf16, tag="identb")
            bq = res.tile([128, 3 * nh], f32, tag="bq")
            nc.scalar.dma_start(trimaskT, tm_d)
            nc.scalar.dma_start(identb, id_d)
            nc.scalar.dma_start(bq, bq_d)

            wt = {}

            def issue_w(h, eng=None):
                for p in range(3):
                    t = w_p.tile([128, D], bf16, tag="w", name=f"w{h}_{p}")
                    r0 = (p * nh + h) * 128
                    (eng or nc.sync).dma_start(t, wq_d[r0:r0 + 128, :])
                    wt[(h, p)] = t

            issue_w(0, eng=nc.scalar)

            # ---- x resident as one big tile; panel-major (span-major) DMA
            # so qkv(0) span 0 can start before the rest of x lands.
            # Panel 0 is split across the SP and ACT issue queues. ----
            xT = res.tile([128, KT * S], bf16, tag="xT")
            xT3 = xT.rearrange("p (kt s) -> p kt s", kt=KT)

            def xs(kt):
                return xT3[:, kt, :]

            for sp in range(NS):
                g = 2 if sp == 0 else 4
                g = min(g, KT)
                for gi, k0 in enumerate(range(0, KT, g)):
                    src = x_d[k0 * 128:(k0 + g) * 128,
                              sp * span:(sp + 1) * span]
                    eng = nc.scalar if (sp == 0 and gi % 2 == 1) else nc.sync
                    eng.dma_start(
                        xT3[:, k0:k0 + g, sp * span:(sp + 1) * span],
                        src.rearrange("(kt p) s -> p kt s", p=128))

            # ---- persistent per-head outputs ----
            OT = [res.tile([128, S], bf16, tag=f"ot{h}", name=f"ot{h}")
                  for h in range(nh)]
            dsts = {}
            vblk = {}
            wpt = []

            def qkv_units(h):
                """Units of one span x one projection (KT matmuls + bias)."""
                if h + 1 < nh:
                    issue_w(h + 1)
                if h == nh - 1:
                    for g in range(nh):
                        t = wp_p.tile([128, D], bf16, tag="wpt",
                                      name=f"wp{g}")
                        nc.sync.dma_start(t, wp_d[g * 128:(g + 1) * 128, :])
                        wpt.append(t)
                QT = qk_p.tile([128, S], bf16, tag="qt", name=f"qt{h}")
                KTt = qk_p.tile([128, S], bf16, tag="kt_", name=f"ktt{h}")
                VT = qk_p.tile([128, S], bf16, tag="vt", name=f"vt{h}")
                vh = v_p.tile([128, S], bf16, tag="v", name=f"vh{h}")
                dsts[h] = (QT, KTt)
                vblk[h] = vh
                dst3 = (QT, KTt, VT)
                for sp in range(NS):
                    for p in range(3):
                        ps = ps_mm.tile([128, span], f32, tag="mm")
                        w = wt[(h, p)]
                        for kt in range(KT):
                            nc.tensor.matmul(
                                ps, w[:, kt * 128:(kt + 1) * 128],
                                xs(kt)[:, sp * span:(sp + 1) * span],
                                start=(kt == 0), stop=(kt == KT - 1))
                        hp = p * nh + h
                        nc.scalar.activation(
                            dst3[p][:, sp * span:(sp + 1) * span], ps,
                            Act.Identity, bias=bq[:, hp:hp + 1], scale=1.0)
                        if p == 2:
                            # V natural layout via PE transpose
                            pst = ps_st.tile([128, span], bf16, tag="st",
                                             name=f"pst{h}_{sp}")
                            for j in range(KPS):
                                tb = sp * KPS + j
                                nc.tensor.transpose(
                                    pst[:, j * 128:(j + 1) * 128],
                                    VT[:, tb * 128:(tb + 1) * 128], identb)
                            nc.vector.tensor_copy(
                                out=vh[:, sp * span:(sp + 1) * span], in_=pst)
                        yield (16, None)

            def attn_units(h):
                """Units of one k-block (score mm + exp + acc; lagged AV mm).
                Yields (weight, completed_span|None)."""
                QT, KTt = dsts[h]
                vh = vblk[h]
                for sp in range(NS):
                    nkj = KPS * (sp + 1)
                    po = ps_o.tile([128, span], f32, tag="o")
                    acc = sm_p.tile([128, span], f32, tag="acc")
                    pend = []

                    def av(it):
                        kj, pt, qoff = it
                        nc.tensor.matmul(
                            po[:, qoff:], vh[:, kj * 128:(kj + 1) * 128],
                            pt[:, qoff:],
                            start=(kj == 0), stop=(kj == nkj - 1))

                    for kj in range(nkj):
                        while len(pend) > LAG:
                            av(pend.pop(0))
                        qoff = max(0, kj - KPS * sp) * 128
                        ps = ps_st.tile([128, span], f32, tag="st")
                        nc.tensor.matmul(
                            ps[:, qoff:], KTt[:, kj * 128:(kj + 1) * 128],
                            QT[:, sp * span + qoff:(sp + 1) * span],
                            start=True, stop=True)
                        if kj >= KPS * sp:  # diagonal block: causal mask
                            nc.vector.tensor_tensor(
                                out=ps[:, qoff:qoff + 128],
                                in0=ps[:, qoff:qoff + 128],
                                in1=trimaskT, op=Alu.add)
                        pt = pt_p.tile([128, span], bf16, tag="pt")
                        nc.scalar.activation(
                            pt[:, qoff:], ps[:, qoff:], Act.Exp, scale=scale)
                        if kj == 0:
                            nc.vector.tensor_copy(out=acc, in_=pt)
                        else:
                            nc.vector.tensor_tensor(
                                out=acc[:, qoff:], in0=acc[:, qoff:],
                                in1=pt[:, qoff:], op=Alu.add)
                        pend.append((kj, pt, qoff))
                        yield (2, None)
                    while pend:
                        av(pend.pop(0))
                    # softmax denominator entirely off the PE
                    sumb = sm_p.tile([128, span], f32, tag="sum")
                    nc.gpsimd.partition_all_reduce(
                        sumb, acc, channels=128,
                        reduce_op=bass_isa.ReduceOp.add)
                    recipb = sm_p.tile([128, span], f32, tag="recipb")
                    nc.vector.reciprocal_approx_fast(out=recipb, in_=sumb)
                    nc.vector.tensor_tensor(
                        out=OT[h][:, sp * span:(sp + 1) * span],
                        in0=po, in1=recipb, op=Alu.mult)
                    yield (1, sp)

            def drive_pair(ga, atot, gb, btot):
                """Weighted-fair merge of two unit generators."""
                aw = bw = 0.0
                adone = bdone = False
                while not (adone and bdone):
                    pick_a = (not adone) and (bdone or aw * btot <= bw * atot)
                    g = ga if pick_a else gb
                    try:
                        w, _ = next(g)
                        if pick_a:
                            aw += w
                        else:
                            bw += w
                    except StopIteration:
                        if pick_a:
                            adone = True
                        else:
                            bdone = True

            # ---- pipeline driver ----
            for _ in qkv_units(0):
                pass
            qkv_tot = 16 * 3 * NS
            attn_tot = 2 * (KPS * NS * (NS + 1) // 2) + NS
            # inflate attn's total so its stream leads and its ACT/GPSIMD
            # tails finish under the dense matmul work
            attn_lead = attn_tot * 5 // 4
            for h in range(1, nh):
                drive_pair(qkv_units(h), qkv_tot, attn_units(h - 1),
                           attn_lead)

            # ---- final phase: proj interleaved with attn(nh-1),
            # gated so proj of span sp waits for attn span sp ----
            punits = [(sp, dcp, half) for sp in range(NS)
                      for dcp in range(DC // 2) for half in range(2)]
            yc_cur = [None]

            def emit_proj(sp, dcp, half):
                if half == 0:
                    yc_cur[0] = yc_p.tile([128, 2 * span], f32, tag="yc",
                                          name=f"yc{sp}_{dcp}")
                yc = yc_cur[0]
                dc = dcp * 2 + half
                ps = ps_mm.tile([128, span], f32, tag="mm")
                for g in range(nh):
                    nc.tensor.matmul(
                        ps, wpt[g][:, dc * 128:(dc + 1) * 128],
                        OT[g][:, sp * span:(sp + 1) * span],
                        start=(g == 0), stop=(g == nh - 1))
                if half == 0:
                    nc.scalar.copy(yc[:, :span], ps)
                else:
                    nc.vector.tensor_copy(out=yc[:, span:], in_=ps)
                    dst = yt_d[dcp * 256:(dcp + 1) * 256,
                               sp * span:(sp + 1) * span]
                    nc.sync.dma_start(
                        dst.rearrange("(two p) s -> p two s", p=128),
                        yc.rearrange("p (two s) -> p two s", two=2))

            ag = attn_units(nh - 1)
            aw = pw = 0.0
            adone = False
            span_done = -1
            pi = 0
            ptot = 4 * len(punits)
            while (not adone) or pi < len(punits):
                can_p = pi < len(punits) and punits[pi][0] <= span_done
                pick_a = (not adone) and (
                    not can_p or aw * ptot <= pw * attn_lead)
                if pick_a:
                    try:
                        w, m = next(ag)
                        aw += w
                        if m is not None:
                            span_done = m
                    except StopIteration:
                        adone = True
                elif can_p:
                    emit_proj(*punits[pi])
                    pi += 1
                    pw += 4
                else:
                    # attn exhausted but gate not open: shouldn't happen
                    raise AssertionError("proj gating stuck")

    nc.finalize()
    return nc


def _prep_core_inputs(x, W_qkv, b_qkv, W_proj, core, S=S, D=D, nh=HPC):
    import ml_dtypes
    bf16 = ml_dtypes.bfloat16
    ngr = NCORES // B
    b, hg = core // ngr, core % ngr
    KT = D // 128
    Dfull = W_qkv.shape[0]

    wq = np.empty((3 * nh * 128, D), dtype=bf16)
    bq = np.zeros((128, 3 * nh), dtype=np.float32)
    for p in range(3):
        for h in range(nh):
            g = hg * nh + h
            col = p * Dfull + g * 128
            blk = W_qkv[:, col:col + 128]            # [D, 128]
            hp = p * nh + h
            wq[hp * 128:(hp + 1) * 128] = (
                blk.reshape(KT, 128, 128).transpose(1, 0, 2).reshape(128, D)
                .astype(bf16))
            bq[:, hp] = b_qkv[col:col + 128]
    wp = W_proj[hg * nh * 128:(hg + 1) * nh * 128, :].astype(bf16)

    r = np.arange(128)
    trimaskT = np.where(r[:, None] <= r[None, :], 0.0, NEG).astype(np.float32)
    return {
        "xt": np.ascontiguousarray(x[b].T).astype(bf16),
        "wqkv": wq,
        "bqkv": bq,
        "wproj": wp,
        "trimaskT": trimaskT,
        "identb": np.eye(128, dtype=bf16),
    }


_CACHE = {}


def kernel(x, W_qkv, b_qkv, W_proj, b_proj, mask):
    from concourse.bass_utils import run_bass_kernel_spmd

    x = np.asarray(x)
    W_qkv = np.asarray(W_qkv)
    b_qkv = np.asarray(b_qkv)
    W_proj = np.asarray(W_proj)
    b_proj = np.asarray(b_proj)

    if "nc" not in _CACHE:
        _CACHE["nc"] = build_nc()
    nc = _CACHE["nc"]

    in_maps = [_prep_core_inputs(x, W_qkv, b_qkv, W_proj, c)
               for c in range(NCORES)]
    res = run_bass_kernel_spmd(nc, in_maps, core_ids=list(range(NCORES)))

    ngr = NCORES // B
    out = np.empty((B, S, D), dtype=np.float32)
    for b in range(B):
        acc = res.results[b * ngr]["yt"].astype(np.float32)
        for g in range(1, ngr):
            acc = acc + res.results[b * ngr + g]["yt"]
        out[b] = acc.T + b_proj[None, :]
    return out


# revision 15
# speedup vs baseline: 1.1418x; 1.0334x over previous
"""Causal self-attention Trainium2 kernel (8 NeuronCores, bf16 compute).

Sharding: core c -> batch b = c//4, head group hg = c%4 (4 heads each).
Each core computes its heads' QKV projections, causal attention, and a
partial output projection yt[d, t] (transposed). Host sums the 4 partials
per batch, transposes, and adds b_proj.

Device dataflow per core (software-pipelined across heads):
  qkv(h) : per span/proj: PSUM = W.T @ xT chunks -> ACT bias -> QT/KT/VT
           VT 128-blocks transposed to natural V via DMA XBAR transpose
  attn(h): per q-span (512): for each k-block kj:
             ST[k,q] = KT_blk.T @ QT_span   (PE, scores transposed)
             += causal mask on diagonal blocks (DVE)
             PT = exp(scale*ST)             (ACT, bf16, unnormalized)
             acc[128,q] += PT               (DVE, f32)
             OT[hd,q] += V_blk.T @ PT       (PE, lagged)
           sum = partition_all_reduce(acc)  (GPSIMD)
           recip (DVE), OT_sbuf = OT * recip (DVE, bf16)
  proj   : yt[dc,t] = sum_h Wp_h.T @ OT_h -> chunked DMA out
  Interleave: attn(h-1) units are woven between qkv(h) units, and
  attn(3) between proj units, so the ACT-bound exp chain never stalls
  the PE.
"""
import numpy as np

B, S, D, H = 2, 2048, 2048, 16
HD = 128
NCORES = 8
HPC = H // (NCORES // B)     # heads per core = 4
NEG = -1e9


def build_nc(S=S, D=D, nh=HPC, span=512):
    import concourse.mybir as mybir
    from concourse import bacc
    from concourse import bass_isa
    from concourse.tile import TileContext

    f32 = mybir.dt.float32
    bf16 = mybir.dt.bfloat16
    KT = D // 128          # contraction tiles for qkv
    TT = S // 128          # token tiles
    NS = S // span         # q spans
    KPS = span // 128      # k-blocks per span
    DC = D // 128
    scale = float(HD) ** -0.5
    LAG = 2

    nc = bacc.Bacc("TRN2", target_bir_lowering=False, debug=False)
    x_d = nc.dram_tensor("xt", [D, S], bf16, kind="ExternalInput").ap()
    wq_d = nc.dram_tensor("wqkv", [3 * nh * 128, D], bf16,
                          kind="ExternalInput").ap()
    bq_d = nc.dram_tensor("bqkv", [128, 3 * nh], f32, kind="ExternalInput").ap()
    wp_d = nc.dram_tensor("wproj", [nh * 128, D], bf16,
                          kind="ExternalInput").ap()
    tm_d = nc.dram_tensor("trimaskT", [128, 128], f32,
                          kind="ExternalInput").ap()
    id_d = nc.dram_tensor("identb", [128, 128], bf16, kind="ExternalInput").ap()
    yt_d = nc.dram_tensor("yt", [D, S], f32, kind="ExternalOutput").ap()

    Act = mybir.ActivationFunctionType
    Alu = mybir.AluOpType

    with TileContext(nc) as tc:
        from contextlib import ExitStack
        with ExitStack() as ctx:
            res = ctx.enter_context(tc.tile_pool(name="res", bufs=1))
            w_p = ctx.enter_context(tc.tile_pool(name="w", bufs=6))
            wp_p = ctx.enter_context(tc.tile_pool(name="wp", bufs=nh))
            qk_p = ctx.enter_context(tc.tile_pool(name="qk", bufs=2))
            v_p = ctx.enter_context(tc.tile_pool(name="v", bufs=2))
            pt_p = ctx.enter_context(tc.tile_pool(name="pt", bufs=4))
            sm_p = ctx.enter_context(tc.tile_pool(name="sm", bufs=2))
            yc_p = ctx.enter_context(tc.tile_pool(name="yc", bufs=4))
            ps_mm = ctx.enter_context(
                tc.tile_pool(name="ps_mm", bufs=2, space="PSUM"))
            ps_o = ctx.enter_context(
                tc.tile_pool(name="ps_o", bufs=2, space="PSUM"))
            ps_st = ctx.enter_context(
                tc.tile_pool(name="ps_st", bufs=4, space="PSUM"))

            # constants (issued on the ACT queue; SP queue is for x panels)
            trimaskT = res.tile([128, 128], f32, tag="trimaskT")
            identb = res.tile([128, 128], bf16, tag="identb")
            bq = res.tile([128, 3 * nh], f32, tag="bq")
            nc.scalar.dma_start(trimaskT, tm_d)
            nc.scalar.dma_start(identb, id_d)
            nc.scalar.dma_start(bq, bq_d)

            wt = {}

            def issue_w(h, eng=None):
                for p in range(3):
                    t = w_p.tile([128, D], bf16, tag="w", name=f"w{h}_{p}")
                    r0 = (p * nh + h) * 128
                    (eng or nc.sync).dma_start(t, wq_d[r0:r0 + 128, :])
                    wt[(h, p)] = t

            issue_w(0, eng=nc.scalar)

            # ---- x resident as one big tile; panel-major (span-major) DMA
            # so qkv(0) span 0 can start before the rest of x lands.
            # Panel 0 is split across the SP and ACT issue queues. ----
            xT = res.tile([128, KT * S], bf16, tag="xT")
            xT3 = xT.rearrange("p (kt s) -> p kt s", kt=KT)

            def xs(kt):
                return xT3[:, kt, :]

            for sp in range(NS):
                g = 2 if sp == 0 else 4
                g = min(g, KT)
                for gi, k0 in enumerate(range(0, KT, g)):
                    src = x_d[k0 * 128:(k0 + g) * 128,
                              sp * span:(sp + 1) * span]
                    eng = nc.scalar if (sp == 0 and gi % 2 == 1) else nc.sync
                    eng.dma_start(
                        xT3[:, k0:k0 + g, sp * span:(sp + 1) * span],
                        src.rearrange("(kt p) s -> p kt s", p=128))

            # ---- persistent per-head outputs ----
            OT = [res.tile([128, S], bf16, tag=f"ot{h}", name=f"ot{h}")
                  for h in range(nh)]
            dsts = {}
            vblk = {}
            wpt = []

            def qkv_units(h):
                """Units of one span x one projection (KT matmuls + bias)."""
                if h + 1 < nh:
                    issue_w(h + 1)
                if h == nh - 1:
                    for g in range(nh):
                        t = wp_p.tile([128, D], bf16, tag="wpt",
                                      name=f"wp{g}")
                        nc.sync.dma_start(t, wp_d[g * 128:(g + 1) * 128, :])
                        wpt.append(t)
                QT = qk_p.tile([128, S], bf16, tag="qt", name=f"qt{h}")
                KTt = qk_p.tile([128, S], bf16, tag="kt_", name=f"ktt{h}")
                VT = qk_p.tile([128, S], bf16, tag="vt", name=f"vt{h}")
                vh = v_p.tile([128, S], bf16, tag="v", name=f"vh{h}")
                dsts[h] = (QT, KTt)
                vblk[h] = vh
                dst3 = (QT, KTt, VT)
                for sp in range(NS):
                    for p in range(3):
                        ps = ps_mm.tile([128, span], f32, tag="mm")
                        w = wt[(h, p)]
                        for kt in range(KT):
                            nc.tensor.matmul(
                                ps, w[:, kt * 128:(kt + 1) * 128],
                                xs(kt)[:, sp * span:(sp + 1) * span],
                                start=(kt == 0), stop=(kt == KT - 1))
                        hp = p * nh + h
                        nc.scalar.activation(
                            dst3[p][:, sp * span:(sp + 1) * span], ps,
                            Act.Identity, bias=bq[:, hp:hp + 1], scale=1.0)
                        if p == 2:
                            # V natural layout via PE transpose
                            pst = ps_st.tile([128, span], bf16, tag="st",
                                             name=f"pst{h}_{sp}")
                            for j in range(KPS):
                                tb = sp * KPS + j
                                nc.tensor.transpose(
                                    pst[:, j * 128:(j + 1) * 128],
                                    VT[:, tb * 128:(tb + 1) * 128], identb)
                            nc.vector.tensor_copy(
                                out=vh[:, sp * span:(sp + 1) * span], in_=pst)
                        yield ("unit", 16)
                    yield ("done", ("qkv", h, sp))

            def attn_units(h):
                """Units of one k-block (score mm + exp + acc; lagged AV mm).
                Span sp is gated on qkv(h) having emitted span sp."""
                yield ("req", ("qkv", h, 0))
                QT, KTt = dsts[h]
                vh = vblk[h]
                for sp in range(NS):
                    if sp > 0:
                        yield ("req", ("qkv", h, sp))
                    nkj = KPS * (sp + 1)
                    po = ps_o.tile([128, span], f32, tag="o")
                    acc = sm_p.tile([128, span], f32, tag="acc")
                    pend = []

                    def av(it):
                        kj, pt, qoff = it
                        nc.tensor.matmul(
                            po[:, qoff:], vh[:, kj * 128:(kj + 1) * 128],
                            pt[:, qoff:],
                            start=(kj == 0), stop=(kj == nkj - 1))

                    for kj in range(nkj):
                        while len(pend) > LAG:
                            av(pend.pop(0))
                        qoff = max(0, kj - KPS * sp) * 128
                        ps = ps_st.tile([128, span], f32, tag="st")
                        nc.tensor.matmul(
                            ps[:, qoff:], KTt[:, kj * 128:(kj + 1) * 128],
                            QT[:, sp * span + qoff:(sp + 1) * span],
                            start=True, stop=True)
                        if kj >= KPS * sp:  # diagonal block: causal mask
                            nc.vector.tensor_tensor(
                                out=ps[:, qoff:qoff + 128],
                                in0=ps[:, qoff:qoff + 128],
                                in1=trimaskT, op=Alu.add)
                        pt = pt_p.tile([128, span], bf16, tag="pt")
                        nc.scalar.activation(
                            pt[:, qoff:], ps[:, qoff:], Act.Exp, scale=scale)
                        if kj == 0:
                            nc.vector.tensor_copy(out=acc, in_=pt)
                        else:
                            nc.vector.tensor_tensor(
                                out=acc[:, qoff:], in0=acc[:, qoff:],
                                in1=pt[:, qoff:], op=Alu.add)
                        pend.append((kj, pt, qoff))
                        yield ("unit", 2)
                    while pend:
                        av(pend.pop(0))
                    # softmax denominator entirely off the PE
                    sumb = sm_p.tile([128, span], f32, tag="sum")
                    nc.gpsimd.partition_all_reduce(
                        sumb, acc, channels=128,
                        reduce_op=bass_isa.ReduceOp.add)
                    recipb = sm_p.tile([128, span], f32, tag="recipb")
                    nc.vector.reciprocal_approx_fast(out=recipb, in_=sumb)
                    nc.vector.tensor_tensor(
                        out=OT[h][:, sp * span:(sp + 1) * span],
                        in0=po, in1=recipb, op=Alu.mult)
                    yield ("unit", 1)
                    yield ("done", ("attn", h, sp))

            def proj_units():
                """Output projection; span sp gated on attn(nh-1) span sp."""
                for sp in range(NS):
                    yield ("req", ("attn", nh - 1, sp))
                    for dcp in range(DC // 2):
                        yc = yc_p.tile([128, 2 * span], f32, tag="yc",
                                       name=f"yc{sp}_{dcp}")
                        for half in range(2):
                            dc = dcp * 2 + half
                            ps = ps_mm.tile([128, span], f32, tag="mm")
                            for g in range(nh):
                                nc.tensor.matmul(
                                    ps, wpt[g][:, dc * 128:(dc + 1) * 128],
                                    OT[g][:, sp * span:(sp + 1) * span],
                                    start=(g == 0), stop=(g == nh - 1))
                            if half == 0:
                                nc.scalar.copy(yc[:, :span], ps)
                            else:
                                nc.vector.tensor_copy(out=yc[:, span:],
                                                      in_=ps)
                                dst = yt_d[dcp * 256:(dcp + 1) * 256,
                                           sp * span:(sp + 1) * span]
                                nc.sync.dma_start(
                                    dst.rearrange("(two p) s -> p two s",
                                                  p=128),
                                    yc.rearrange("p (two s) -> p two s",
                                                 two=2))
                            yield ("unit", 4)

            # ---- unified dependency-gated two-stream scheduler ----
            def chain(gens):
                for g in gens:
                    yield from g

            def drive(dense, dtot, attn, atot, lead=1.25):
                streams = [
                    {"g": dense, "tot": float(dtot), "w": 0.0,
                     "req": None, "done": False},
                    {"g": attn, "tot": float(atot) * lead, "w": 0.0,
                     "req": None, "done": False},
                ]
                state = set()
                while True:
                    cands = [s for s in streams if not s["done"] and
                             (s["req"] is None or s["req"] in state)]
                    if not cands:
                        if all(s["done"] for s in streams):
                            return
                        raise AssertionError("scheduler deadlock")
                    s = min(cands, key=lambda s: s["w"] / s["tot"])
                    s["req"] = None
                    while True:
                        try:
                            item = next(s["g"])
                        except StopIteration:
                            s["done"] = True
                            break
                        kind = item[0]
                        if kind == "unit":
                            s["w"] += item[1]
                            break
                        elif kind == "done":
                            state.add(item[1])
                        elif kind == "req":
                            if item[1] not in state:
                                s["req"] = item[1]
                                break

            dense_tot = 16 * 3 * NS * nh + 4 * DC * NS
            attn_tot = (2 * (KPS * NS * (NS + 1) // 2) + NS) * nh
            drive(chain([qkv_units(h) for h in range(nh)] + [proj_units()]),
                  dense_tot,
                  chain([attn_units(h) for h in range(nh)]),
                  attn_tot)

    nc.finalize()
    return nc


def _prep_core_inputs(x, W_qkv, b_qkv, W_proj, core, S=S, D=D, nh=HPC):
    import ml_dtypes
    bf16 = ml_dtypes.bfloat16
    ngr = NCORES // B
    b, hg = core // ngr, core % ngr
    KT = D // 128
    Dfull = W_qkv.shape[0]

    wq = np.empty((3 * nh * 128, D), dtype=bf16)
    bq = np.zeros((128, 3 * nh), dtype=np.float32)
    for p in range(3):
        for h in range(nh):
            g = hg * nh + h
            col = p * Dfull + g * 128
            blk = W_qkv[:, col:col + 128]            # [D, 128]
            hp = p * nh + h
            wq[hp * 128:(hp + 1) * 128] = (
                blk.reshape(KT, 128, 128).transpose(1, 0, 2).reshape(128, D)
                .astype(bf16))
            bq[:, hp] = b_qkv[col:col + 128]
    wp = W_proj[hg * nh * 128:(hg + 1) * nh * 128, :].astype(bf16)

    r = np.arange(128)
    trimaskT = np.where(r[:, None] <= r[None, :], 0.0, NEG).astype(np.float32)
    return {
        "xt": np.ascontiguousarray(x[b].T).astype(bf16),
        "wqkv": wq,
        "bqkv": bq,
        "wproj": wp,
        "trimaskT": trimaskT,
        "identb": np.eye(128, dtype=bf16),
    }


_CACHE = {}


def kernel(x, W_qkv, b_qkv, W_proj, b_proj, mask):
    from concourse.bass_utils import run_bass_kernel_spmd

    x = np.asarray(x)
    W_qkv = np.asarray(W_qkv)
    b_qkv = np.asarray(b_qkv)
    W_proj = np.asarray(W_proj)
    b_proj = np.asarray(b_proj)

    if "nc" not in _CACHE:
        _CACHE["nc"] = build_nc()
    nc = _CACHE["nc"]

    in_maps = [_prep_core_inputs(x, W_qkv, b_qkv, W_proj, c)
               for c in range(NCORES)]
    res = run_bass_kernel_spmd(nc, in_maps, core_ids=list(range(NCORES)))

    ngr = NCORES // B
    out = np.empty((B, S, D), dtype=np.float32)
    for b in range(B):
        acc = res.results[b * ngr]["yt"].astype(np.float32)
        for g in range(1, ngr):
            acc = acc + res.results[b * ngr + g]["yt"]
        out[b] = acc.T + b_proj[None, :]
    return out
